# revision 8
# baseline (speedup 1.0000x reference)
"""Trainium2 Bass kernel for nn_Divergence2d.

Math (from the reference):
  q = C//4 = 4 channel groups A=x[:, :4], B=x[:,4:8], C=x[:,8:12], D=x[:,12:16]
  With per-group channel sums  A(r,c) = sum_ch lam_ch x[ch, r, c]  (lam only
  for group A) and a padded map  Gpad[r, c] = G[r-2, c-2]:

    out1[i,j] = (Apad[i+1, j+2] - Apad[i+1, j]) + Bpad[i, j+1] - Bpad[i+2, j+1]
    out2[i,j] = (Cpad[i+1, j+2] - Cpad[i+1, j]) + Dpad[i, j+1] - Dpad[i+2, j+1]

  for i,j in [0, 514)  (lam applied in the DVE combine when all lams equal).

Strategy: pure data parallel, 2 images per core on 8 cores.  Per image the
514 output rows are split into blocks of 126; per block ONE HWDGE DMA loads
a row window of all 16 channels into an SBUF tile [rows, 16ch x 512].  The
TensorE does the stencil via stationary matrices (weights [window_row,
out_row], channel sum by PSUM accumulation over 4 chained matmuls).  All
matmuls run in float32r (TF32-like) mode: 1 cycle/row vs 4 for fp32, the
loose rel-err budget (2e-2) dwarfs the ~1e-3 rounding.

Vertical conv padding is folded into the weights instead of rhs memsets:
  - block 0 loads x rows [0:128) at partition 0 and uses row-shifted
    weights (taps at negative x rows simply have no weight entry);
  - the last block (10 out rows) contracts only K=10 loaded partitions, so
    taps at x rows >= 512 drop out of the contraction.
ScalarE (ACT) drains PSUM into zero-padded SBUF staging tiles (pad columns
memset only on each staging buffer's first use); DVE does 4 combine ops per
block; one HWDGE DMA stores each block.
"""
import sys

for _p in (
    "/root/.axon_site",
    "/root/.axon_site/_ro/trn_rl_repo",
    "/root/.axon_site/_ro/pypackages",
    "/opt/trn_rl_repo",
):
    if _p not in sys.path:
        sys.path.append(_p)

import numpy as np

N_CORES = 8
N, C, H, W = 16, 16, 512, 512
PB = N // N_CORES          # images per core
HO = WO = H + 2            # 514
CWPAD = 516                # staging width (2-col pad each side)
BLK = 126                  # output rows per block (matmul M)
BLOCKS = []
_i0 = 0
while _i0 < HO:
    BLOCKS.append((_i0, min(BLK, HO - _i0)))
    _i0 += BLK
# -> [(0,126), (126,126), (252,126), (378,126), (504,10)]

_cache = {}


def _build(lam4):
    import concourse.bacc as bacc
    import concourse.mybir as mybir
    from concourse.tile import TileContext

    f32 = mybir.dt.float32
    f32r = mybir.dt.float32r
    ALU = mybir.AluOpType
    ACT_COPY = mybir.ActivationFunctionType.Copy
    lam_eq = all(float(v) == float(lam4[0]) for v in lam4)

    nc = bacc.Bacc("TRN2", target_bir_lowering=False, debug=False,
                   num_devices=N_CORES, detect_race_conditions=False)
    x = nc.dram_tensor("x", (PB, C, H, W), f32, kind="ExternalInput")
    out = nc.dram_tensor("out", (PB, 2, HO, WO), f32, kind="ExternalOutput")

    with TileContext(nc) as tc:
        with (
            tc.tile_pool(name="consts", bufs=1) as c_pool,
            tc.tile_pool(name="rhs", bufs=4) as rhs_pool,
            tc.tile_pool(name="psum", bufs=2, space="PSUM") as ps_pool,
            tc.tile_pool(name="stage", bufs=2) as st_pool,
            tc.tile_pool(name="outs", bufs=3) as out_pool,
            tc.tile_pool(name="dtmp", bufs=2) as d_pool,
        ):
            # ---- one-time stencil weights [128 window rows, 126 out rows] --
            # Interior blocks (window row r = x row i0-2+r, out local m):
            #   S_s1[r, m] = d(r, m+1)           (A/C tap at x row i-1)
            #   S_bd[r, m] = d(r, m) - d(r, m+2) (B/D rows i-2 / i)
            # Block 0 (window row r = x row r):
            #   S_s1f[r, m] = d(r, m-1)
            #   S_bdf[r, m] = d(r, m-2) - d(r, m)
            def iota_t(tag, base, mult):
                t = c_pool.tile([128, BLK], f32, tag=tag, name=tag)
                nc.gpsimd.iota(t[:, :], pattern=[[0 if mult else 1, BLK]],
                               base=base, channel_multiplier=mult,
                               allow_small_or_imprecise_dtypes=True)
                return t

            R0 = iota_t("R0", 0, 1)          # r
            R1 = iota_t("R1", 1, 1)          # r + 1
            R2 = iota_t("R2", 2, 1)          # r + 2
            Sm0 = iota_t("Sm0", 0, 0)        # m
            Sm1 = iota_t("Sm1", 1, 0)        # m + 1
            Sm2 = iota_t("Sm2", 2, 0)        # m + 2

            def eq_t(tag, a, b, dt=None):
                t = c_pool.tile([128, BLK], dt or f32, tag=tag, name=tag)
                nc.vector.tensor_tensor(t[:, :], a[:, :], b[:, :], ALU.is_equal)
                return t

            # final weight tiles are float32r so their producing instruction
            # "rounds" them for the fp32r matmuls (values are exact anyway)
            S_s1 = eq_t("S_s1", R0, Sm1, f32r)
            e0 = eq_t("e0", R0, Sm0)         # (r == m)
            e2 = eq_t("e2", R0, Sm2)         # (r == m+2)
            S_bd = c_pool.tile([128, BLK], f32r, tag="S_bd")
            nc.vector.tensor_tensor(S_bd[:, :], e0[:, :], e2[:, :], ALU.subtract)
            S_s1f = eq_t("S_s1f", R1, Sm0, f32r)   # (r == m-1)
            e0f = eq_t("e0f", R2, Sm0)       # (r == m-2)
            S_bdf = c_pool.tile([128, BLK], f32r, tag="S_bdf")
            nc.vector.tensor_tensor(S_bdf[:, :], e0f[:, :], e0[:, :], ALU.subtract)

            if lam_eq:
                S_A_per_ch = [S_s1] * 4      # lam applied in the combine
                S_Af_per_ch = [S_s1f] * 4
            else:
                S_A_per_ch, S_Af_per_ch = [], []
                for c4 in range(4):
                    t = c_pool.tile([128, BLK], f32r, tag=f"S_A{c4}",
                                    name=f"S_A{c4}")
                    nc.vector.tensor_scalar_mul(t[:, :], S_s1[:, :],
                                                float(lam4[c4]))
                    S_A_per_ch.append(t)
                    tf = c_pool.tile([128, BLK], f32r, tag=f"S_Af{c4}",
                                     name=f"S_Af{c4}")
                    nc.vector.tensor_scalar_mul(tf[:, :], S_s1f[:, :],
                                                float(lam4[c4]))
                    S_Af_per_ch.append(tf)

            # ---- main loop ---------------------------------------------
            blk_idx = 0
            for n in range(PB):
                for i0, nr in BLOCKS:
                    blk_idx += 1
                    first = (i0 == 0)
                    if first:
                        rlo, K = 0, 128          # x rows [0:128) at part 0
                    else:
                        rlo = i0 - 2             # window row r = x row rlo+r
                        K = min(128, H - rlo)    # 128, or 10 for last block
                    t = rhs_pool.tile([128, 16 * 512], f32r, tag="rhs")
                    tv = t[:, :].rearrange("p (c w) -> p c w", w=512)
                    if blk_idx == 1:
                        # split a small head off the very first DMA so the
                        # queues start draining before the full descriptor
                        # block is generated
                        nc.sync.dma_start(out=tv[0:16, :, :],
                                          in_=x[n, :, 0:16, :].rearrange(
                                              "c r w -> r c w").bitcast(f32r))
                        nc.sync.dma_start(out=tv[16:K, :, :],
                                          in_=x[n, :, 16:K, :].rearrange(
                                              "c r w -> r c w").bitcast(f32r))
                    else:
                        nc.sync.dma_start(out=tv[0:K, :, :],
                                          in_=x[n, :, rlo:rlo + K, :].rearrange(
                                              "c r w -> r c w").bitcast(f32r))
                    # group order A,B then C,D: the out1 combine only needs
                    # maps 0/1, so DVE overlaps the second half of the matmuls
                    if first:
                        groups = [(0, S_Af_per_ch), (1, [S_bdf] * 4),
                                  (2, [S_s1f] * 4), (3, [S_bdf] * 4)]
                    else:
                        groups = [(0, S_A_per_ch), (1, [S_bd] * 4),
                                  (2, [S_s1] * 4), (3, [S_bd] * 4)]
                    ps = {}
                    for g, weights in groups:
                        p = ps_pool.tile([128, 512], f32, tag=f"ps{g}",
                                         name=f"ps{g}")
                        ps[g] = p
                        for c4 in range(4):
                            ch = 4 * g + c4
                            nc.tensor.matmul(
                                p[0:BLK, :],
                                weights[c4][0:K, :],
                                t[0:K, 512 * ch:512 * ch + 512],
                                start=(c4 == 0), stop=(c4 == 3))
                    # ACT drains PSUM into zero-padded staging tiles; the pad
                    # columns are only memset on each buffer's first use
                    # (st_pool bufs=2 -> blocks 1 and 2 prime both buffers)
                    prime = blk_idx <= 2
                    st = {}
                    for g in range(4):
                        s = st_pool.tile([128, CWPAD], f32, tag=f"st{g}",
                                         name=f"st{g}")
                        st[g] = s
                        if g in (0, 2):   # A/C: data at cols [2:514)
                            if prime:
                                nc.vector.memset(s[:, 0:2], 0.0)
                                nc.vector.memset(s[:, 514:CWPAD], 0.0)
                            nc.scalar.activation(s[0:nr, 2:514], ps[g][0:nr, :],
                                                 ACT_COPY)
                        else:             # B/D: data at cols [1:513)
                            if prime:
                                nc.vector.memset(s[:, 0:1], 0.0)
                                nc.vector.memset(s[:, 513:CWPAD], 0.0)
                            nc.scalar.activation(s[0:nr, 1:513], ps[g][0:nr, :],
                                                 ACT_COPY)
                    # DVE combine: 4 ops per block
                    o = out_pool.tile([128, 2 * WO], f32, tag="o")
                    dA = d_pool.tile([128, WO], f32, tag="dA")
                    dC = d_pool.tile([128, WO], f32, tag="dC")
                    nc.vector.tensor_tensor(dA[0:nr, :], st[0][0:nr, 2:2 + WO],
                                            st[0][0:nr, 0:WO], ALU.subtract)
                    if lam_eq:
                        nc.vector.scalar_tensor_tensor(
                            o[0:nr, 0:WO], dA[0:nr, :], float(lam4[0]),
                            st[1][0:nr, 0:WO], ALU.mult, ALU.add)
                    else:
                        nc.vector.tensor_tensor(o[0:nr, 0:WO], dA[0:nr, :],
                                                st[1][0:nr, 0:WO], ALU.add)
                    nc.vector.tensor_tensor(dC[0:nr, :], st[2][0:nr, 2:2 + WO],
                                            st[2][0:nr, 0:WO], ALU.subtract)
                    nc.vector.tensor_tensor(o[0:nr, WO:2 * WO], dC[0:nr, :],
                                            st[3][0:nr, 0:WO], ALU.add)
                    # store
                    osrc = o[0:nr, :].rearrange("p (ch w) -> p ch w", w=WO)
                    ov = out[n].rearrange("ch r w -> r ch w")
                    nc.scalar.dma_start(out=ov[i0:i0 + nr, :, :], in_=osrc)
    nc.finalize()
    return nc


def _get_nc(lam4):
    key = tuple(float(v) for v in lam4)
    if key not in _cache:
        _cache[key] = _build(key)
    return _cache[key]


def _run(xs: np.ndarray, lam4, trace: bool = False, tmpdir=None):
    from concourse.bass_utils import run_bass_kernel_spmd

    nc = _get_nc(lam4)
    in_maps = [{"x": np.ascontiguousarray(xs[PB * c:PB * (c + 1)])}
               for c in range(N_CORES)]
    res = run_bass_kernel_spmd(nc, in_maps, list(range(N_CORES)),
                               trace=trace, tmpdir=tmpdir)
    full = np.concatenate([res.results[c]["out"] for c in range(N_CORES)], axis=0)
    return full, res


def kernel(x, lam1x, lam2x, lam1y, lam2y):
    x = np.ascontiguousarray(np.asarray(x, dtype=np.float32))
    assert x.shape == (N, C, H, W), x.shape
    lam4 = np.asarray(lam1x, dtype=np.float32).reshape(-1)
    assert lam4.shape == (4,), lam4.shape
    full, _ = _run(x, lam4)
    return full


# revision 10
# speedup vs baseline: 1.0765x; 1.0765x over previous
"""Trainium2 Bass kernel for nn_Divergence2d.

Math (from the reference):
  q = C//4 = 4 channel groups A=x[:, :4], B=x[:,4:8], C=x[:,8:12], D=x[:,12:16]
  With per-group channel sums  A(r,c) = sum_ch lam_ch x[ch, r, c]  (lam only
  for group A) and a padded map  Gpad[r, c] = G[r-2, c-2]:

    out1[i,j] = (Apad[i+1, j+2] - Apad[i+1, j]) + Bpad[i, j+1] - Bpad[i+2, j+1]
    out2[i,j] = (Cpad[i+1, j+2] - Cpad[i+1, j]) + Dpad[i, j+1] - Dpad[i+2, j+1]

  for i,j in [0, 514)  (lam applied in the DVE combine when all lams equal).

Strategy: pure data parallel, 2 images per core on 8 cores.  Per image the
514 output rows are split into blocks of 126; per block ONE HWDGE DMA loads
a row window of all 16 channels into an SBUF tile [rows, 16ch x 512].  The
TensorE does the stencil via stationary matrices (weights [window_row,
out_row], channel sum by PSUM accumulation over 4 chained matmuls).  All
matmuls run in float32r (TF32-like) mode: 1 cycle/row vs 4 for fp32, the
loose rel-err budget (2e-2) dwarfs the ~1e-3 rounding.

Vertical conv padding is folded into the weights instead of rhs memsets:
  - block 0 loads x rows [0:128) at partition 0 and uses row-shifted
    weights (taps at negative x rows simply have no weight entry);
  - the last block (10 out rows) contracts only K=10 loaded partitions, so
    taps at x rows >= 512 drop out of the contraction.
ScalarE (ACT) drains PSUM into zero-padded SBUF staging tiles (pad columns
memset only on each staging buffer's first use); DVE does 4 combine ops per
block; one HWDGE DMA stores each block.
"""
import sys

for _p in (
    "/root/.axon_site",
    "/root/.axon_site/_ro/trn_rl_repo",
    "/root/.axon_site/_ro/pypackages",
    "/opt/trn_rl_repo",
):
    if _p not in sys.path:
        sys.path.append(_p)

import numpy as np

N_CORES = 8
N, C, H, W = 16, 16, 512, 512
PB = N // N_CORES          # images per core
HO = WO = H + 2            # 514
CWPAD = 516                # staging width (2-col pad each side)
BLK = 126                  # output rows per block (matmul M)
BLOCKS = []
_i0 = 0
while _i0 < HO:
    BLOCKS.append((_i0, min(BLK, HO - _i0)))
    _i0 += BLK
# -> [(0,126), (126,126), (252,126), (378,126), (504,10)]

_cache = {}


def _build(lam4):
    import concourse.bacc as bacc
    import concourse.mybir as mybir
    from concourse.tile import TileContext

    f32 = mybir.dt.float32
    f32r = mybir.dt.float32r
    ALU = mybir.AluOpType
    ACT_COPY = mybir.ActivationFunctionType.Copy
    lam_eq = all(float(v) == float(lam4[0]) for v in lam4)

    nc = bacc.Bacc("TRN2", target_bir_lowering=False, debug=False,
                   num_devices=N_CORES, detect_race_conditions=False)
    x = nc.dram_tensor("x", (PB, C, H, W), f32, kind="ExternalInput")
    out = nc.dram_tensor("out", (PB, 2, HO, WO), f32, kind="ExternalOutput")

    with TileContext(nc) as tc:
        with (
            tc.tile_pool(name="consts", bufs=1) as c_pool,
            tc.tile_pool(name="rhs", bufs=3) as rhs_pool,
            tc.tile_pool(name="psum", bufs=2, space="PSUM") as ps_pool,
            tc.tile_pool(name="stage", bufs=2) as st_pool,
            tc.tile_pool(name="outs", bufs=3) as out_pool,
            tc.tile_pool(name="dtmp", bufs=2) as d_pool,
        ):
            # ---- one-time stencil weights [128 window rows, 126 out rows] --
            # Interior blocks (window row r = x row i0-2+r, out local m):
            #   S_s1[r, m] = d(r, m+1)           (A/C tap at x row i-1)
            #   S_bd[r, m] = d(r, m) - d(r, m+2) (B/D rows i-2 / i)
            # Block 0 (window row r = x row r):
            #   S_s1f[r, m] = d(r, m-1)
            #   S_bdf[r, m] = d(r, m-2) - d(r, m)
            def iota_t(tag, base, mult):
                t = c_pool.tile([128, BLK], f32, tag=tag, name=tag)
                nc.gpsimd.iota(t[:, :], pattern=[[0 if mult else 1, BLK]],
                               base=base, channel_multiplier=mult,
                               allow_small_or_imprecise_dtypes=True)
                return t

            R0 = iota_t("R0", 0, 1)          # r
            R1 = iota_t("R1", 1, 1)          # r + 1
            R2 = iota_t("R2", 2, 1)          # r + 2
            Sm0 = iota_t("Sm0", 0, 0)        # m
            Sm1 = iota_t("Sm1", 1, 0)        # m + 1
            Sm2 = iota_t("Sm2", 2, 0)        # m + 2

            def eq_t(tag, a, b, dt=None):
                t = c_pool.tile([128, BLK], dt or f32, tag=tag, name=tag)
                nc.vector.tensor_tensor(t[:, :], a[:, :], b[:, :], ALU.is_equal)
                return t

            # final weight tiles are float32r so their producing instruction
            # "rounds" them for the fp32r matmuls (values are exact anyway)
            S_s1 = eq_t("S_s1", R0, Sm1, f32r)
            e0 = eq_t("e0", R0, Sm0)         # (r == m)
            e2 = eq_t("e2", R0, Sm2)         # (r == m+2)
            S_bd = c_pool.tile([128, BLK], f32r, tag="S_bd")
            nc.vector.tensor_tensor(S_bd[:, :], e0[:, :], e2[:, :], ALU.subtract)
            S_s1f = eq_t("S_s1f", R1, Sm0, f32r)   # (r == m-1)
            e0f = eq_t("e0f", R2, Sm0)       # (r == m-2)
            S_bdf = c_pool.tile([128, BLK], f32r, tag="S_bdf")
            nc.vector.tensor_tensor(S_bdf[:, :], e0f[:, :], e0[:, :], ALU.subtract)

            if lam_eq:
                S_A_per_ch = [S_s1] * 4      # lam applied in the combine
                S_Af_per_ch = [S_s1f] * 4
            else:
                S_A_per_ch, S_Af_per_ch = [], []
                for c4 in range(4):
                    t = c_pool.tile([128, BLK], f32r, tag=f"S_A{c4}",
                                    name=f"S_A{c4}")
                    nc.vector.tensor_scalar_mul(t[:, :], S_s1[:, :],
                                                float(lam4[c4]))
                    S_A_per_ch.append(t)
                    tf = c_pool.tile([128, BLK], f32r, tag=f"S_Af{c4}",
                                     name=f"S_Af{c4}")
                    nc.vector.tensor_scalar_mul(tf[:, :], S_s1f[:, :],
                                                float(lam4[c4]))
                    S_Af_per_ch.append(tf)

            # ---- main loop ---------------------------------------------
            blk_idx = 0
            for n in range(PB):
                for i0, nr in BLOCKS:
                    blk_idx += 1
                    first = (i0 == 0)
                    if first:
                        rlo, K = 0, 128          # x rows [0:128) at part 0
                    else:
                        rlo = i0 - 2             # window row r = x row rlo+r
                        K = min(128, H - rlo)    # 128, or 10 for last block
                    t = rhs_pool.tile([128, 16 * 512], f32r, tag="rhs")
                    tv = t[:, :].rearrange("p (c w) -> p c w", w=512)
                    nc.sync.dma_start(out=tv[0:K, :, :],
                                      in_=x[n, :, rlo:rlo + K, :].rearrange(
                                          "c r w -> r c w").bitcast(f32r))
                    # group order A,B then C,D: the out1 combine only needs
                    # maps 0/1, so DVE overlaps the second half of the matmuls
                    if first:
                        groups = [(0, S_Af_per_ch), (1, [S_bdf] * 4),
                                  (2, [S_s1f] * 4), (3, [S_bdf] * 4)]
                    else:
                        groups = [(0, S_A_per_ch), (1, [S_bd] * 4),
                                  (2, [S_s1] * 4), (3, [S_bd] * 4)]
                    ps = {}
                    for g, weights in groups:
                        p = ps_pool.tile([128, 512], f32, tag=f"ps{g}",
                                         name=f"ps{g}")
                        ps[g] = p
                        for c4 in range(4):
                            ch = 4 * g + c4
                            nc.tensor.matmul(
                                p[0:BLK, :],
                                weights[c4][0:K, :],
                                t[0:K, 512 * ch:512 * ch + 512],
                                start=(c4 == 0), stop=(c4 == 3))
                    # ACT drains PSUM into zero-padded staging tiles; the pad
                    # columns are only memset on each buffer's first use
                    # (st_pool bufs=2 -> blocks 1 and 2 prime both buffers)
                    prime = blk_idx <= 2
                    st = {}
                    for g in range(4):
                        s = st_pool.tile([128, CWPAD], f32, tag=f"st{g}",
                                         name=f"st{g}")
                        st[g] = s
                        if g in (0, 2):   # A/C: data at cols [2:514)
                            if prime:
                                nc.vector.memset(s[:, 0:2], 0.0)
                                nc.vector.memset(s[:, 514:CWPAD], 0.0)
                            nc.scalar.activation(s[0:nr, 2:514], ps[g][0:nr, :],
                                                 ACT_COPY)
                        else:             # B/D: data at cols [1:513)
                            if prime:
                                nc.vector.memset(s[:, 0:1], 0.0)
                                nc.vector.memset(s[:, 513:CWPAD], 0.0)
                            nc.scalar.activation(s[0:nr, 1:513], ps[g][0:nr, :],
                                                 ACT_COPY)
                    # DVE combine: 4 ops per block
                    o = out_pool.tile([128, 2 * WO], f32, tag="o")
                    dA = d_pool.tile([128, WO], f32, tag="dA")
                    dC = d_pool.tile([128, WO], f32, tag="dC")
                    nc.vector.tensor_tensor(dA[0:nr, :], st[0][0:nr, 2:2 + WO],
                                            st[0][0:nr, 0:WO], ALU.subtract)
                    if lam_eq:
                        nc.vector.scalar_tensor_tensor(
                            o[0:nr, 0:WO], dA[0:nr, :], float(lam4[0]),
                            st[1][0:nr, 0:WO], ALU.mult, ALU.add)
                    else:
                        nc.vector.tensor_tensor(o[0:nr, 0:WO], dA[0:nr, :],
                                                st[1][0:nr, 0:WO], ALU.add)
                    nc.vector.tensor_tensor(dC[0:nr, :], st[2][0:nr, 2:2 + WO],
                                            st[2][0:nr, 0:WO], ALU.subtract)
                    nc.vector.tensor_tensor(o[0:nr, WO:2 * WO], dC[0:nr, :],
                                            st[3][0:nr, 0:WO], ALU.add)
                    # store
                    osrc = o[0:nr, :].rearrange("p (ch w) -> p ch w", w=WO)
                    ov = out[n].rearrange("ch r w -> r ch w")
                    nc.scalar.dma_start(out=ov[i0:i0 + nr, :, :], in_=osrc)
    nc.finalize()
    return nc


def _get_nc(lam4):
    key = tuple(float(v) for v in lam4)
    if key not in _cache:
        _cache[key] = _build(key)
    return _cache[key]


def _run(xs: np.ndarray, lam4, trace: bool = False, tmpdir=None):
    from concourse.bass_utils import run_bass_kernel_spmd

    nc = _get_nc(lam4)
    in_maps = [{"x": np.ascontiguousarray(xs[PB * c:PB * (c + 1)])}
               for c in range(N_CORES)]
    res = run_bass_kernel_spmd(nc, in_maps, list(range(N_CORES)),
                               trace=trace, tmpdir=tmpdir)
    full = np.concatenate([res.results[c]["out"] for c in range(N_CORES)], axis=0)
    return full, res


def kernel(x, lam1x, lam2x, lam1y, lam2y):
    x = np.ascontiguousarray(np.asarray(x, dtype=np.float32))
    assert x.shape == (N, C, H, W), x.shape
    lam4 = np.asarray(lam1x, dtype=np.float32).reshape(-1)
    assert lam4.shape == (4,), lam4.shape
    full, _ = _run(x, lam4)
    return full


# revision 12
# speedup vs baseline: 1.0893x; 1.0119x over previous
"""Trainium2 Bass kernel for nn_Divergence2d.

Math (from the reference):
  q = C//4 = 4 channel groups A=x[:, :4], B=x[:,4:8], C=x[:,8:12], D=x[:,12:16]
  With per-group channel sums  A(r,c) = sum_ch lam_ch x[ch, r, c]  (lam only
  for group A) and a padded map  Gpad[r, c] = G[r-2, c-2]:

    out1[i,j] = (Apad[i+1, j+2] - Apad[i+1, j]) + Bpad[i, j+1] - Bpad[i+2, j+1]
    out2[i,j] = (Cpad[i+1, j+2] - Cpad[i+1, j]) + Dpad[i, j+1] - Dpad[i+2, j+1]

  for i,j in [0, 514)  (lam applied in the DVE combine when all lams equal).

Strategy: pure data parallel, 2 images per core on 8 cores.  Per image the
514 output rows are split into blocks of 126; per block ONE HWDGE DMA loads
a row window of all 16 channels into an SBUF tile [rows, 16ch x 512].  The
TensorE does the stencil via stationary matrices (weights [window_row,
out_row], channel sum by PSUM accumulation over 4 chained matmuls).  All
matmuls run in float32r (TF32-like) mode: 1 cycle/row vs 4 for fp32, the
loose rel-err budget (2e-2) dwarfs the ~1e-3 rounding.

Vertical conv padding is folded into the weights instead of rhs memsets:
  - block 0 loads x rows [0:128) at partition 0 and uses row-shifted
    weights (taps at negative x rows simply have no weight entry);
  - the last block (10 out rows) contracts only K=10 loaded partitions, so
    taps at x rows >= 512 drop out of the contraction.
ScalarE (ACT) drains PSUM into zero-padded SBUF staging tiles (pad columns
memset only on each staging buffer's first use); DVE does 4 combine ops per
block; one HWDGE DMA stores each block.
"""
import sys

for _p in (
    "/root/.axon_site",
    "/root/.axon_site/_ro/trn_rl_repo",
    "/root/.axon_site/_ro/pypackages",
    "/opt/trn_rl_repo",
):
    if _p not in sys.path:
        sys.path.append(_p)

import numpy as np

N_CORES = 8
N, C, H, W = 16, 16, 512, 512
PB = N // N_CORES          # images per core
HO = WO = H + 2            # 514
CWPAD = 516                # staging width (2-col pad each side)
BLK = 126                  # output rows per block (matmul M)
BLOCKS = []
_i0 = 0
while _i0 < HO:
    BLOCKS.append((_i0, min(BLK, HO - _i0)))
    _i0 += BLK
# -> [(0,126), (126,126), (252,126), (378,126), (504,10)]

_cache = {}


def _build(lam4):
    import concourse.bacc as bacc
    import concourse.mybir as mybir
    from concourse.tile import TileContext

    f32 = mybir.dt.float32
    f32r = mybir.dt.float32r
    ALU = mybir.AluOpType
    ACT_COPY = mybir.ActivationFunctionType.Copy
    lam_eq = all(float(v) == float(lam4[0]) for v in lam4)

    nc = bacc.Bacc("TRN2", target_bir_lowering=False, debug=False,
                   num_devices=N_CORES, detect_race_conditions=False)
    x = nc.dram_tensor("x", (PB, C, H, W), f32, kind="ExternalInput")
    out = nc.dram_tensor("out", (PB, 2, HO, WO), f32, kind="ExternalOutput")

    with TileContext(nc) as tc:
        with (
            tc.tile_pool(name="consts", bufs=1) as c_pool,
            tc.tile_pool(name="rhs", bufs=3) as rhs_pool,
            tc.tile_pool(name="psum", bufs=2, space="PSUM") as ps_pool,
            tc.tile_pool(name="stage", bufs=2) as st_pool,
            tc.tile_pool(name="outs", bufs=3) as out_pool,
            tc.tile_pool(name="dtmp", bufs=2) as d_pool,
        ):
            # ---- one-time stencil weights [128 window rows, 126 out rows] --
            # Interior blocks (window row r = x row i0-2+r, out local m):
            #   S_s1[r, m] = d(r, m+1)           (A/C tap at x row i-1)
            #   S_bd[r, m] = d(r, m) - d(r, m+2) (B/D rows i-2 / i)
            # Block 0 (window row r = x row r):
            #   S_s1f[r, m] = d(r, m-1)
            #   S_bdf[r, m] = d(r, m-2) - d(r, m)
            def iota_t(tag, base, mult):
                t = c_pool.tile([128, BLK], f32, tag=tag, name=tag)
                nc.gpsimd.iota(t[:, :], pattern=[[0 if mult else 1, BLK]],
                               base=base, channel_multiplier=mult,
                               allow_small_or_imprecise_dtypes=True)
                return t

            R0 = iota_t("R0", 0, 1)          # r
            R1 = iota_t("R1", 1, 1)          # r + 1
            R2 = iota_t("R2", 2, 1)          # r + 2
            Sm0 = iota_t("Sm0", 0, 0)        # m
            Sm1 = iota_t("Sm1", 1, 0)        # m + 1
            Sm2 = iota_t("Sm2", 2, 0)        # m + 2

            def eq_t(tag, a, b, dt=None):
                t = c_pool.tile([128, BLK], dt or f32, tag=tag, name=tag)
                nc.vector.tensor_tensor(t[:, :], a[:, :], b[:, :], ALU.is_equal)
                return t

            # final weight tiles are float32r so their producing instruction
            # "rounds" them for the fp32r matmuls (values are exact anyway)
            S_s1 = eq_t("S_s1", R0, Sm1, f32r)
            e0 = eq_t("e0", R0, Sm0)         # (r == m)
            e2 = eq_t("e2", R0, Sm2)         # (r == m+2)
            S_bd = c_pool.tile([128, BLK], f32r, tag="S_bd")
            nc.vector.tensor_tensor(S_bd[:, :], e0[:, :], e2[:, :], ALU.subtract)
            S_s1f = eq_t("S_s1f", R1, Sm0, f32r)   # (r == m-1)
            e0f = eq_t("e0f", R2, Sm0)       # (r == m-2)
            S_bdf = c_pool.tile([128, BLK], f32r, tag="S_bdf")
            nc.vector.tensor_tensor(S_bdf[:, :], e0f[:, :], e0[:, :], ALU.subtract)

            if lam_eq:
                S_A_per_ch = [S_s1] * 4      # lam applied in the combine
                S_Af_per_ch = [S_s1f] * 4
            else:
                S_A_per_ch, S_Af_per_ch = [], []
                for c4 in range(4):
                    t = c_pool.tile([128, BLK], f32r, tag=f"S_A{c4}",
                                    name=f"S_A{c4}")
                    nc.vector.tensor_scalar_mul(t[:, :], S_s1[:, :],
                                                float(lam4[c4]))
                    S_A_per_ch.append(t)
                    tf = c_pool.tile([128, BLK], f32r, tag=f"S_Af{c4}",
                                     name=f"S_Af{c4}")
                    nc.vector.tensor_scalar_mul(tf[:, :], S_s1f[:, :],
                                                float(lam4[c4]))
                    S_Af_per_ch.append(tf)

            # ---- main loop ---------------------------------------------
            blk_idx = 0
            for n in range(PB):
                for i0, nr in BLOCKS:
                    blk_idx += 1
                    first = (i0 == 0)
                    if first:
                        rlo, K = 0, 128          # x rows [0:128) at part 0
                    else:
                        rlo = i0 - 2             # window row r = x row rlo+r
                        K = min(128, H - rlo)    # 128, or 10 for last block
                    t = rhs_pool.tile([128, 16 * 512], f32r, tag="rhs")
                    tv = t[:, :].rearrange("p (c w) -> p c w", w=512)
                    if n == PB - 1 and i0 >= BLOCKS[-2][0]:
                        # last two blocks: split by channel halves so the
                        # final compute chain starts before the whole block
                        # is resident (shorter end-of-kernel drain)
                        nc.sync.dma_start(out=tv[0:K, 0:8, :],
                                          in_=x[n, 0:8, rlo:rlo + K, :].rearrange(
                                              "c r w -> r c w").bitcast(f32r))
                        nc.sync.dma_start(out=tv[0:K, 8:16, :],
                                          in_=x[n, 8:16, rlo:rlo + K, :].rearrange(
                                              "c r w -> r c w").bitcast(f32r))
                    else:
                        nc.sync.dma_start(out=tv[0:K, :, :],
                                          in_=x[n, :, rlo:rlo + K, :].rearrange(
                                              "c r w -> r c w").bitcast(f32r))
                    # group order A,B then C,D: the out1 combine only needs
                    # maps 0/1, so DVE overlaps the second half of the matmuls
                    if first:
                        groups = [(0, S_Af_per_ch), (1, [S_bdf] * 4),
                                  (2, [S_s1f] * 4), (3, [S_bdf] * 4)]
                    else:
                        groups = [(0, S_A_per_ch), (1, [S_bd] * 4),
                                  (2, [S_s1] * 4), (3, [S_bd] * 4)]
                    ps = {}
                    for g, weights in groups:
                        p = ps_pool.tile([128, 512], f32, tag=f"ps{g}",
                                         name=f"ps{g}")
                        ps[g] = p
                        for c4 in range(4):
                            ch = 4 * g + c4
                            nc.tensor.matmul(
                                p[0:BLK, :],
                                weights[c4][0:K, :],
                                t[0:K, 512 * ch:512 * ch + 512],
                                start=(c4 == 0), stop=(c4 == 3))
                    # ACT drains the A/C maps into zero-padded staging tiles
                    # (pad columns memset only on each buffer's first use);
                    # B/D are read directly from PSUM in the DVE combine
                    prime = blk_idx <= 2
                    st = {}
                    for g in (0, 2):      # A/C: data at cols [2:514)
                        s = st_pool.tile([128, CWPAD], f32, tag=f"st{g}",
                                         name=f"st{g}")
                        st[g] = s
                        if prime:
                            nc.vector.memset(s[:, 0:2], 0.0)
                            nc.vector.memset(s[:, 514:CWPAD], 0.0)
                        nc.scalar.activation(s[0:nr, 2:514], ps[g][0:nr, :],
                                             ACT_COPY)
                    # DVE combine
                    o = out_pool.tile([128, 2 * WO], f32, tag="o")
                    dA = d_pool.tile([128, WO], f32, tag="dA")
                    nc.vector.tensor_tensor(dA[0:nr, :], st[0][0:nr, 2:2 + WO],
                                            st[0][0:nr, 0:WO], ALU.subtract)
                    if lam_eq:
                        lam0 = float(lam4[0])
                        nc.vector.tensor_scalar_mul(o[0:nr, 0:1],
                                                    dA[0:nr, 0:1], lam0)
                        nc.vector.tensor_scalar_mul(o[0:nr, 513:514],
                                                    dA[0:nr, 513:514], lam0)
                        nc.vector.scalar_tensor_tensor(
                            o[0:nr, 1:513], dA[0:nr, 1:513], lam0,
                            ps[1][0:nr, :], ALU.mult, ALU.add)
                    else:
                        nc.vector.tensor_scalar_mul(o[0:nr, 0:1],
                                                    dA[0:nr, 0:1], 1.0)
                        nc.vector.tensor_scalar_mul(o[0:nr, 513:514],
                                                    dA[0:nr, 513:514], 1.0)
                        nc.vector.tensor_tensor(o[0:nr, 1:513],
                                                dA[0:nr, 1:513],
                                                ps[1][0:nr, :], ALU.add)
                    nc.vector.tensor_tensor(o[0:nr, WO:2 * WO],
                                            st[2][0:nr, 2:2 + WO],
                                            st[2][0:nr, 0:WO], ALU.subtract)
                    nc.vector.tensor_tensor(o[0:nr, WO + 1:WO + 513],
                                            o[0:nr, WO + 1:WO + 513],
                                            ps[3][0:nr, :], ALU.add)
                    # store
                    osrc = o[0:nr, :].rearrange("p (ch w) -> p ch w", w=WO)
                    ov = out[n].rearrange("ch r w -> r ch w")
                    nc.scalar.dma_start(out=ov[i0:i0 + nr, :, :], in_=osrc)
    nc.finalize()
    return nc


def _get_nc(lam4):
    key = tuple(float(v) for v in lam4)
    if key not in _cache:
        _cache[key] = _build(key)
    return _cache[key]


def _run(xs: np.ndarray, lam4, trace: bool = False, tmpdir=None):
    from concourse.bass_utils import run_bass_kernel_spmd

    nc = _get_nc(lam4)
    in_maps = [{"x": np.ascontiguousarray(xs[PB * c:PB * (c + 1)])}
               for c in range(N_CORES)]
    res = run_bass_kernel_spmd(nc, in_maps, list(range(N_CORES)),
                               trace=trace, tmpdir=tmpdir)
    full = np.concatenate([res.results[c]["out"] for c in range(N_CORES)], axis=0)
    return full, res


def kernel(x, lam1x, lam2x, lam1y, lam2y):
    x = np.ascontiguousarray(np.asarray(x, dtype=np.float32))
    assert x.shape == (N, C, H, W), x.shape
    lam4 = np.asarray(lam1x, dtype=np.float32).reshape(-1)
    assert lam4.shape == (4,), lam4.shape
    full, _ = _run(x, lam4)
    return full
